# revision 3
# baseline (speedup 1.0000x reference)
"""Trainium2 Bass kernel for nn_BankedMergeHeads.

Math (per token t, slot k):
    out[t] = sum_k p[t,k] * (x[t,k] @ W[sel[t,k]] + b[sel[t,k]])

Strategy (8 NeuronCores = 4-way d_model x 2-way tokens):
  - Each core owns a (token-half, d_model-quarter): 1024 tokens x 512 cols.
  - Host-side routing ("dispatch"): sort the core's 4096 (token,slot) pairs
    by selected bank, pad each bank segment to a uniform per-bank capacity C
    (same C on all cores -> identical SPMD kernel IR; data-dependence lives
    only in input arrays).
  - Device grouped GEMM (orientation A): stationary = X^T piece (<=128
    pairs), moving = W[bank] column-slice (128x512, float32r full-rate),
    PSUM out = proj (pairs x 512) fp32.
  - Evict PSUM -> SBUF fp16 (split across ScalarE/VectorE), DMA to a DRAM
    staging buffer (contiguous 128-row blocks).
  - dma_gather permutes staging rows into token-major order (the MoE
    "combine" all-to-all, on-device).
  - Merge = PE matmuls: per 128-token chunk, PSUM accumulates one bias
    matmul (routing matrix PB^T @ b-slice) + four block-diagonal
    probability matmuls (P4^T @ gathered rows) via tile_position col tiles.
    The probabilities p live in the host-built routing matrices P4/PB
    (pure routing metadata x gate values); all x/W/b arithmetic is on-device.
  - Evict fp32 out chunks, DMA out; host reassembles the full output.
"""

import sys

import numpy as np

sys.path.insert(0, "/opt/trn_rl_repo")

# Problem constants (hardcoded per task contract).
B, S, K = 2, 1024, 4
NUM_BANKS = 32
D_HEAD = 128
D_MODEL = 2048
N_CORES = 8
DM, TK = 4, 2                # d_model split x token split
NT_L = (B * S) // TK         # tokens per core (1024)
NP_L = NT_L * K              # pairs per core (4096)
DMC = D_MODEL // DM          # cols per core (512)
NG = NT_L // 32              # 32-token merge groups per core
TCH = NT_L // 128            # 128-token chunks per core

_CACHE = {}


def _build_nc(C, gemm_dtype_name):
    """Build the SPMD Bass kernel. C = per-bank padded capacity (mult of 32)."""
    import concourse.bacc as bacc
    import concourse.mybir as mybir
    import concourse.tile as tile

    f32 = mybir.dt.float32
    f32r = mybir.dt.float32r
    fp16 = mybir.dt.float16
    i16 = mybir.dt.int16
    gdt = {"f32r": f32r, "fp16": fp16, "f32": f32}[gemm_dtype_name]

    NPAD = NUM_BANKS * C                    # padded pair rows
    assert NPAD % 128 == 0

    nc = bacc.Bacc("TRN2", target_bir_lowering=False, debug=False,
                   num_devices=N_CORES)
    XT_d = nc.dram_tensor("XT", [D_HEAD, NPAD], gdt, kind="ExternalInput")
    W_d = nc.dram_tensor("Wq", [D_HEAD, NUM_BANKS * DMC], gdt,
                         kind="ExternalInput")
    gidx_d = nc.dram_tensor("gidx", [128, NP_L // 16], i16,
                            kind="ExternalInput")
    P4_d = nc.dram_tensor("P4", [128, NG * 32], fp16, kind="ExternalInput")
    PB_d = nc.dram_tensor("PB", [NUM_BANKS, NT_L], fp16, kind="ExternalInput")
    bT_d = nc.dram_tensor("bT", [NUM_BANKS, DMC], fp16, kind="ExternalInput")
    out_d = nc.dram_tensor("out", [NT_L, DMC], f32, kind="ExternalOutput")
    scratch_d = nc.dram_tensor("scratch", [NPAD, DMC], fp16)

    # per-bank psum pieces of <=128 pairs
    pieces = []
    for n in range(NUM_BANKS):
        off = 0
        while off < C:
            m = min(128, C - off)
            pieces.append((n, off, m))
            off += m

    with tile.TileContext(nc) as tc:
        with tc.tile_pool(name="inp", bufs=1) as inp, \
             tc.tile_pool(name="ppg", bufs=5, space="PSUM") as ppg, \
             tc.tile_pool(name="ppm", bufs=3, space="PSUM") as ppm, \
             tc.tile_pool(name="ev", bufs=6) as ev, \
             tc.tile_pool(name="big", bufs=1) as big, \
             tc.tile_pool(name="ob", bufs=4) as ob:
            XT = inp.tile([D_HEAD, NPAD], gdt)
            nc.sync.dma_start(XT[:], XT_d.ap())
            Wq = inp.tile([D_HEAD, NUM_BANKS * DMC], gdt)
            nc.sync.dma_start(Wq[:], W_d.ap())
            gidx = inp.tile([128, NP_L // 16], i16)
            nc.sync.dma_start(gidx[:], gidx_d.ap())
            P4 = inp.tile([128, NG * 32], fp16)
            nc.sync.dma_start(P4[:], P4_d.ap())
            PB = inp.tile([NUM_BANKS, NT_L], fp16)
            nc.sync.dma_start(PB[:], PB_d.ap())
            bT = inp.tile([NUM_BANKS, DMC], fp16)
            nc.sync.dma_start(bT[:], bT_d.ap())

            # ---- grouped GEMM + evict + stage-out ----
            evict_flip = 0
            for (n, off, m) in pieces:
                ps = ppg.tile([128, DMC], mybir.dt.float32, tag="ps")
                nc.tensor.matmul(
                    ps[:m, :],
                    lhsT=XT[:, n * C + off: n * C + off + m],
                    rhs=Wq[:, n * DMC:(n + 1) * DMC],
                    start=True, stop=True)
                st = ev.tile([128, DMC], mybir.dt.float16, tag="st")
                if evict_flip == 0:
                    nc.scalar.copy(st[:m, :], ps[:m, :])
                else:
                    nc.vector.tensor_copy(st[:m, :], ps[:m, :])
                evict_flip ^= 1
                # contiguous DRAM block for these m rows
                nc.sync.dma_start(
                    scratch_d.ap()[n * C + off: n * C + off + m, :], st[:m, :])

            # ---- permute to token-major via dma_gather ----
            merged = big.tile([128, NP_L // 128, DMC], mybir.dt.float16)
            nc.gpsimd.dma_gather(
                out_ap=merged[:], in_ap=scratch_d.ap(), idxs_ap=gidx[:],
                num_idxs=NP_L, num_idxs_reg=NP_L, elem_size=DMC,
                single_packet=False)

            # ---- merge: bias matmul + 4 prob matmuls per 128-token chunk ----
            for t in range(TCH):
                po = ppm.tile([128, DMC], mybir.dt.float32, tag="po")
                nc.tensor.matmul(
                    po[:], lhsT=PB[:, t * 128:(t + 1) * 128], rhs=bT[:],
                    start=True, stop=False)
                for j in range(4):
                    g = t * 4 + j
                    nc.tensor.matmul(
                        po[32 * j:32 * (j + 1), :],
                        lhsT=P4[:, g * 32:(g + 1) * 32],
                        rhs=merged[:, g, :],
                        start=False, stop=(j == 3),
                        tile_position=(0, 32 * j))
                osb = ob.tile([128, DMC], mybir.dt.float32, tag="osb")
                if t % 2 == 0:
                    nc.scalar.copy(osb[:], po[:])
                else:
                    nc.vector.tensor_copy(osb[:], po[:])
                nc.sync.dma_start(out_d.ap()[t * 128:(t + 1) * 128, :], osb[:])

    nc.compile()
    return nc


def _prepare(tensor, head_selection, head_probabilities, W, b, C=None,
             gemm_dtype_name="f32r"):
    """Host-side sharding + routing metadata. Returns (in_maps, C)."""
    x = np.asarray(tensor, dtype=np.float32).reshape(B * S, K, D_HEAD)
    sel = np.asarray(head_selection).astype(np.int64).reshape(B * S, K)
    p = np.asarray(head_probabilities, dtype=np.float32).reshape(B * S, K)
    Wf = np.asarray(W, dtype=np.float32)
    bf = np.asarray(b, dtype=np.float32)

    halves = []
    maxcount = 0
    for tk in range(TK):
        t0 = tk * NT_L
        sel_h = sel[t0:t0 + NT_L].reshape(-1)          # (NP_L,)
        order = np.argsort(sel_h, kind="stable")        # sorted pair ids
        counts = np.bincount(sel_h, minlength=NUM_BANKS)
        maxcount = max(maxcount, int(counts.max()))
        halves.append((t0, sel_h, order, counts))
    if C is None:
        C = max(160, ((maxcount + 31) // 32) * 32)
    assert C >= maxcount
    NPAD = NUM_BANKS * C

    xdt = np.float16 if gemm_dtype_name == "fp16" else np.float32

    in_maps = [None] * N_CORES
    for tk in range(TK):
        t0, sel_h, order, counts = halves[tk]
        x_h = x[t0:t0 + NT_L].reshape(NP_L, D_HEAD)
        p_h = p[t0:t0 + NT_L]                          # (NT_L, K)

        # padded row position of each sorted pair
        seg_start = np.zeros(NUM_BANKS, dtype=np.int64)
        seg_start[1:] = np.cumsum(counts)[:-1]
        padpos = np.empty(NP_L, dtype=np.int64)
        for n in range(NUM_BANKS):
            s0 = seg_start[n]
            padpos[s0:s0 + counts[n]] = n * C + np.arange(counts[n])
        # pair j (token-major) -> padded row
        row_of_pair = np.empty(NP_L, dtype=np.int64)
        row_of_pair[order] = padpos

        Xpad = np.zeros((NPAD, D_HEAD), dtype=np.float32)
        Xpad[padpos] = x_h[order]
        XT_np = np.ascontiguousarray(Xpad.T).astype(xdt)   # (128, NPAD)

        gidx_np = np.zeros((128, NP_L // 16), dtype=np.int16)
        rows16 = row_of_pair.reshape(NP_L // 16, 16).T.astype(np.int16)
        for g in range(8):
            gidx_np[16 * g:16 * (g + 1), :] = rows16

        P4_np = np.zeros((128, NG * 32), dtype=np.float16)
        pg = p_h.reshape(NG, 32, K)
        for gi in range(NG):
            for i in range(32):
                for k in range(K):
                    P4_np[4 * i + k, gi * 32 + i] = pg[gi, i, k]

        PB_np = np.zeros((NUM_BANKS, NT_L), dtype=np.float32)
        np.add.at(PB_np, (sel_h.reshape(NT_L, K).T,
                          np.broadcast_to(np.arange(NT_L), (K, NT_L))),
                  p_h.T)
        PB_np = PB_np.astype(np.float16)

        for dm in range(DM):
            c0 = dm * DMC
            W_np = np.ascontiguousarray(
                Wf[:, :, c0:c0 + DMC].transpose(1, 0, 2).reshape(
                    D_HEAD, NUM_BANKS * DMC)).astype(xdt)
            bT_np = bf[:, c0:c0 + DMC].astype(np.float16)
            core = tk * DM + dm
            in_maps[core] = {
                "XT": XT_np, "Wq": W_np, "gidx": gidx_np,
                "P4": P4_np, "PB": PB_np, "bT": bT_np,
            }
    return in_maps, C


def _run(tensor, head_selection, head_probabilities, W, b,
         gemm_dtype_name="f32r", trace=False):
    from concourse import bass_utils

    in_maps, C = _prepare(tensor, head_selection, head_probabilities, W, b,
                          gemm_dtype_name=gemm_dtype_name)
    key = (C, gemm_dtype_name)
    if key not in _CACHE:
        _CACHE[key] = _build_nc(C, gemm_dtype_name)
    nc = _CACHE[key]
    res = bass_utils.run_bass_kernel_spmd(
        nc, in_maps, core_ids=list(range(N_CORES)), trace=trace)

    out = np.zeros((B * S, D_MODEL), dtype=np.float32)
    for core in range(N_CORES):
        tk, dm = core // DM, core % DM
        oc = res.results[core]["out"]
        out[tk * NT_L:(tk + 1) * NT_L, dm * DMC:(dm + 1) * DMC] = oc
    return out.reshape(B, S, D_MODEL), res


def kernel(tensor, head_selection, head_probabilities, W, b):
    out, _ = _run(tensor, head_selection, head_probabilities, W, b)
    return out
